# revision 10
# baseline (speedup 1.0000x reference)
"""Trainium2 Bass kernel for DiffusionReturnPrediction.

Data-parallel over batch (B=1024 -> 128/core on 8 cores). Per core:
  phase 1: score-net GEMM1  h = silu(x_flat @ W1 + 0.1*ws + b1)   (bf16)
  phase 2: score-net GEMM2  scores = h @ W2 + b2, scattered (via PE
           transpose) into the LSTM's [feature, (n,b)] input layout
  phase 3: 180-step LSTM over B*N=1792 sequences, H=128
           z kept as [gate_unit(128) x bn] in PSUM, gate math on ACT/DVE
  phase 4: GCN (A baked as immediates) + MLP head + spatial pool

All matmuls bf16 (fp32 PSUM accumulation); final pool matmul fp32.
"""

import numpy as np
import ml_dtypes

import concourse.bacc as bacc
import concourse.bass as bass
import concourse.tile as tile
import concourse.mybir as mybir

BF16 = mybir.dt.bfloat16
F32 = mybir.dt.float32
AF = mybir.ActivationFunctionType

B, T, N, F = 1024, 180, 14, 4
D = N * T * F          # 10080
SNH = 1024
H = 128
G = 128
NOUT = 8
NCORES = 8
BS = B // NCORES       # 128 batch per core
BN = BS * N            # 1792
KT = (D + 1 + 127) // 128   # 79 k-tiles for GEMM1 (incl. ones row)
CH = 448               # LSTM bn-chunk width (4 chunks of 448)
NG2 = 6                # t-groups of 32 (180 -> 6 groups, last partial)


def _bf16(a):
    return np.ascontiguousarray(a, dtype=np.float32).astype(ml_dtypes.bfloat16)


def _f32(a):
    return np.ascontiguousarray(a, dtype=np.float32)


def build_nc(A_np, reps=1, t_steps=None, dumps=()):
    """Build + compile the per-core Bass program. A_np: [14,14] f32 dense
    normalized adjacency (baked as immediates)."""
    if t_steps is None:
        t_steps = T
    nc = bacc.Bacc(None, target_bir_lowering=False)
    dump_es = {}
    for dn, dshape, ddt in (
        ("d_hsn", [128, SNH], BF16), ("d_hT", [128, SNH], BF16),
        ("d_scT", [128, NG2 * BN], BF16), ("d_xcxt", [128, NG2 * BN], BF16),
        ("d_hbf", [H, BN], BF16), ("d_cbf", [H, BN], BF16),
        ("d_ubf", [G, BN], BF16), ("d_vm", [G // 2, BN], BF16),
        ("d_v1f", [1, BN], F32),
    ):
        if dn in dumps:
            dump_es[dn] = nc.declare_dram_parameter(dn, dshape, ddt, isOutput=True)

    xt_e = nc.declare_dram_parameter("xt", [D + 1, BS], BF16, isOutput=False)
    xcx_e = nc.declare_dram_parameter("xcx", [NG2 * 128, BN], BF16, isOutput=False)
    w1_e = nc.declare_dram_parameter("w1", [D + 1, SNH], BF16, isOutput=False)
    w2_e = nc.declare_dram_parameter("w2", [SNH + 1, D], BF16, isOutput=False)
    whh_e = nc.declare_dram_parameter("whh", [H, 4 * H], BF16, isOutput=False)
    wihx_e = nc.declare_dram_parameter("wihx", [128, 8 * 512], BF16, isOutput=False)
    wihs_e = nc.declare_dram_parameter("wihs", [128, 8 * 512], BF16, isOutput=False)
    bg_e = nc.declare_dram_parameter("bg", [H, 4], F32, isOutput=False)
    gcnw_e = nc.declare_dram_parameter("gcnw", [H, G], BF16, isOutput=False)
    gcnb_e = nc.declare_dram_parameter("gcnb", [G, 1], F32, isOutput=False)
    mlpw1_e = nc.declare_dram_parameter("mlpw1", [G, G // 2], BF16, isOutput=False)
    mlpb1_e = nc.declare_dram_parameter("mlpb1", [G // 2, 1], F32, isOutput=False)
    mlpw2_e = nc.declare_dram_parameter("mlpw2", [G // 2, 1], BF16, isOutput=False)
    mlpb2_e = nc.declare_dram_parameter("mlpb2", [1, 1], F32, isOutput=False)
    poolw_e = nc.declare_dram_parameter("poolw", [N + 1, NOUT], F32, isOutput=False)
    ident_e = nc.declare_dram_parameter("ident", [128, 128], BF16, isOutput=False)
    out_e = nc.declare_dram_parameter("out", [BS, NOUT], F32, isOutput=True)

    with tile.TileContext(nc) as tc:
        with tc.tile_pool(name="const", bufs=1) as cp:
            identt = cp.tile([128, 128], BF16)
            nc.sync.dma_start(identt[:], ident_e[:])
            bgt = cp.tile([H, 4], F32)
            nc.sync.dma_start(bgt[:], bg_e[:])
            gcnwt = cp.tile([H, G], BF16)
            nc.sync.dma_start(gcnwt[:], gcnw_e[:])
            gcnbt = cp.tile([G, 1], F32)
            nc.sync.dma_start(gcnbt[:], gcnb_e[:])
            mlpw1t = cp.tile([G, G // 2], BF16)
            nc.sync.dma_start(mlpw1t[:], mlpw1_e[:])
            mlpb1t = cp.tile([G // 2, 1], F32)
            nc.sync.dma_start(mlpb1t[:], mlpb1_e[:])
            mlpw2t = cp.tile([G // 2, 1], BF16)
            nc.sync.dma_start(mlpw2t[:], mlpw2_e[:])
            mlpb2t = cp.tile([1, 1], F32)
            nc.sync.dma_start(mlpb2t[:], mlpb2_e[:])
            poolwt = cp.tile([N + 1, NOUT], F32)
            nc.sync.dma_start(poolwt[:], poolw_e[:])
            whht = cp.tile([H, 4 * H], BF16)
            nc.sync.dma_start(whht[:], whh_e[:])
            wihxt = cp.tile([128, 8 * 512], BF16)
            wihst = cp.tile([128, 8 * 512], BF16)
            nc.sync.dma_start(wihxt[:], wihx_e[:])
            nc.sync.dma_start(wihst[:], wihs_e[:])
            ones1 = cp.tile([1, BS], BF16)
            nc.vector.memset(ones1[:], 1.0)

            # resident big tensors
            xcxt = cp.tile([128, NG2 * BN], BF16)   # x part of LSTM input
            for g2 in range(NG2):
                nc.sync.dma_start(xcxt[:, g2 * BN:(g2 + 1) * BN],
                                  xcx_e[g2 * 128:(g2 + 1) * 128, :])
            scT = cp.tile([128, NG2 * BN], BF16)    # scores part (device-filled)
            nc.gpsimd.memset(scT[:], 0.0)
            hT = cp.tile([128, SNH], BF16)          # transposed score-net hidden
            hbf = cp.tile([H, BN], BF16)            # LSTM h state
            cbf = cp.tile([H, BN], BF16)            # LSTM c state

            for _rep in range(reps):
                # ---------------- phase 1: GEMM1 ----------------
                with tc.tile_pool(name="p1", bufs=3) as p1, \
                     tc.tile_pool(name="ps1", bufs=1, space="PSUM") as ps1, \
                     tc.tile_pool(name="ps1t", bufs=2, space="PSUM") as ps1t:
                    xts = p1.tile([128, KT * 128], BF16, tag="xts")
                    for k in range(KT):
                        rows = min(128, D + 1 - k * 128)
                        nc.sync.dma_start(xts[0:rows, k * 128:k * 128 + 128],
                                          xt_e[k * 128:k * 128 + rows, :])
                    hps = ps1.tile([128, SNH], F32)
                    for k in range(KT):
                        rows = min(128, D + 1 - k * 128)
                        w1t = p1.tile([128, SNH], BF16, tag="w1t")
                        nc.sync.dma_start(w1t[0:rows, :],
                                          w1_e[k * 128:k * 128 + rows, :])
                        for jg in range(2):
                            nc.tensor.matmul(
                                hps[:, jg * 512:(jg + 1) * 512],
                                xts[0:rows, k * 128:k * 128 + 128],
                                w1t[0:rows, jg * 512:(jg + 1) * 512],
                                start=(k == 0), stop=(k == KT - 1))
                    hsn = p1.tile([128, SNH], BF16, tag="hsn")
                    nc.scalar.activation(hsn[:], hps[:], AF.Silu)
                    for j in range(8):
                        tp = ps1t.tile([128, 128], BF16, tag="tp1")
                        nc.tensor.transpose(tp[:], hsn[:, j * 128:(j + 1) * 128],
                                            identt[:])
                        nc.vector.tensor_copy(hT[:, j * 128:(j + 1) * 128], tp[:])
                    if "d_hsn" in dump_es:
                        nc.sync.dma_start(dump_es["d_hsn"][:, :], hsn[:])
                    if "d_hT" in dump_es:
                        nc.sync.dma_start(dump_es["d_hT"][:, :], hT[:])

                # ---------------- phase 2: GEMM2 + scatter ----------------
                with tc.tile_pool(name="p2", bufs=3) as p2, \
                     tc.tile_pool(name="ps2", bufs=2, space="PSUM") as ps2, \
                     tc.tile_pool(name="ps2t", bufs=2, space="PSUM") as ps2t:
                    for n in range(N):
                        for (c0, W) in ((0, 256), (256, 256), (512, 208)):
                            w2t = p2.tile([128, 8 * 256], BF16, tag="w2t")
                            for k in range(8):
                                nc.sync.dma_start(
                                    w2t[:, k * 256:k * 256 + W],
                                    w2_e[k * 128:(k + 1) * 128,
                                         n * 720 + c0:n * 720 + c0 + W])
                            w2b = p2.tile([1, 256], BF16, tag="w2b")
                            nc.sync.dma_start(
                                w2b[:, 0:W],
                                w2_e[SNH:SNH + 1, n * 720 + c0:n * 720 + c0 + W])
                            sc = ps2.tile([128, 256], F32, tag="sc")
                            for k in range(8):
                                nc.tensor.matmul(
                                    sc[:, 0:W], hT[:, k * 128:(k + 1) * 128],
                                    w2t[:, k * 256:k * 256 + W],
                                    start=(k == 0), stop=False)
                            nc.tensor.matmul(sc[:, 0:W], ones1[0:1, :],
                                             w2b[0:1, 0:W], start=False, stop=True)
                            sst = p2.tile([128, 256], BF16, tag="sst")
                            nc.scalar.copy(sst[:, 0:W], sc[:, 0:W])
                            for bi in range(2):
                                bw = min(128, W - bi * 128)
                                t0 = (c0 + bi * 128) // 4
                                g2 = t0 // 32
                                tp2 = ps2t.tile([128, 128], BF16, tag="tp2")
                                nc.tensor.transpose(
                                    tp2[0:bw, :],
                                    sst[:, bi * 128:bi * 128 + bw], identt[:])
                                nc.vector.tensor_copy(
                                    scT[0:bw, (g2 * N + n) * 128:
                                        (g2 * N + n) * 128 + 128],
                                    tp2[0:bw, :])

                # ---------------- phase 3: LSTM ----------------
                nc.gpsimd.memset(hbf[:], 0.0)
                nc.gpsimd.memset(cbf[:], 0.0)
                with tc.tile_pool(name="p3", bufs=3) as p3, \
                     tc.tile_pool(name="zp", bufs=1, space="PSUM") as zp:
                    zps = zp.tile([128, 4096], F32)
                    for t in range(t_steps):
                        g2 = t // 32
                        q0 = 32 * ((t % 32) // 8)
                        v = t % 8
                        for half in range(2):
                            cs = (2 * half, 2 * half + 1)
                            # x-part (row-tiled, concurrent across gates)
                            for g in range(4):
                                for c in cs:
                                    st = c % 2
                                    ps = zps[:, (st * 4 + g) * 512:
                                             (st * 4 + g) * 512 + CH]
                                    nc.tensor.matmul(
                                        ps,
                                        wihxt[q0:q0 + 32,
                                              v * 512 + g * 128:
                                              v * 512 + (g + 1) * 128],
                                        xcxt[q0:q0 + 32,
                                             g2 * BN + c * CH:g2 * BN + c * CH + CH],
                                        start=True, stop=False,
                                        tile_position=(q0, 0))
                            # scores part
                            for g in range(4):
                                for c in cs:
                                    st = c % 2
                                    ps = zps[:, (st * 4 + g) * 512:
                                             (st * 4 + g) * 512 + CH]
                                    nc.tensor.matmul(
                                        ps,
                                        wihst[q0:q0 + 32,
                                              v * 512 + g * 128:
                                              v * 512 + (g + 1) * 128],
                                        scT[q0:q0 + 32,
                                            g2 * BN + c * CH:g2 * BN + c * CH + CH],
                                        start=False, stop=False,
                                        tile_position=(q0, 0))
                            # recurrent part
                            for g in range(4):
                                for c in cs:
                                    st = c % 2
                                    ps = zps[:, (st * 4 + g) * 512:
                                             (st * 4 + g) * 512 + CH]
                                    nc.tensor.matmul(
                                        ps, whht[:, g * 128:(g + 1) * 128],
                                        hbf[:, c * CH:c * CH + CH],
                                        start=False, stop=True)
                            # gate math per chunk
                            for c in cs:
                                st = c % 2
                                def zsl(g):
                                    return zps[:, (st * 4 + g) * 512:
                                               (st * 4 + g) * 512 + CH]
                                it = p3.tile([128, CH], BF16, tag="it")
                                ft = p3.tile([128, CH], BF16, tag="ft")
                                gt = p3.tile([128, CH], BF16, tag="gt")
                                ot = p3.tile([128, CH], BF16, tag="ot")
                                nc.scalar.activation(it[:], zsl(0), AF.Sigmoid,
                                                     bias=bgt[:, 0:1])
                                nc.scalar.activation(gt[:], zsl(2), AF.Tanh,
                                                     bias=bgt[:, 2:3])
                                nc.scalar.activation(ft[:], zsl(1), AF.Sigmoid,
                                                     bias=bgt[:, 1:2])
                                nc.scalar.activation(ot[:], zsl(3), AF.Sigmoid,
                                                     bias=bgt[:, 3:4])
                                ig = p3.tile([128, CH], BF16, tag="ig")
                                fc = p3.tile([128, CH], BF16, tag="fc")
                                nc.vector.tensor_mul(ig[:], it[:], gt[:])
                                nc.vector.tensor_mul(fc[:], ft[:],
                                                     cbf[:, c * CH:c * CH + CH])
                                nc.vector.tensor_add(cbf[:, c * CH:c * CH + CH],
                                                     ig[:], fc[:])
                                tct = p3.tile([128, CH], BF16, tag="tct")
                                nc.scalar.activation(tct[:],
                                                     cbf[:, c * CH:c * CH + CH],
                                                     AF.Tanh)
                                nc.vector.tensor_mul(hbf[:, c * CH:c * CH + CH],
                                                     ot[:], tct[:])

                for dn, src in (("d_scT", scT), ("d_xcxt", xcxt),
                                ("d_hbf", hbf), ("d_cbf", cbf)):
                    if dn in dump_es:
                        nc.sync.dma_start(dump_es[dn][:, :], src[:])
                # ---------------- phase 4: GCN + MLP + pool ----------------
                with tc.tile_pool(name="p4", bufs=2) as p4, \
                     tc.tile_pool(name="ps4", bufs=2, space="PSUM") as ps4:
                    ubf = p4.tile([G, BN], BF16, tag="ubf")
                    for c in range(4):
                        ups = ps4.tile([G, CH], F32, tag="ups")
                        nc.tensor.matmul(ups[:], gcnwt[:],
                                         hbf[:, c * CH:c * CH + CH],
                                         start=True, stop=True)
                        nc.vector.tensor_scalar(
                            out=ubf[:, c * CH:c * CH + CH], in0=ups[:],
                            scalar1=gcnbt[:, 0:1], scalar2=None,
                            op0=mybir.AluOpType.add)
                    vbf = p4.tile([G // 2, BN], BF16, tag="vbf")
                    for c in range(4):
                        vps = ps4.tile([G // 2, CH], F32, tag="vps")
                        nc.tensor.matmul(vps[:], mlpw1t[:],
                                         ubf[:, c * CH:c * CH + CH],
                                         start=True, stop=True)
                        nc.scalar.copy(vbf[:, c * CH:c * CH + CH], vps[:])
                    # A-mix over nodes (A baked as immediates, sparse)
                    vm = p4.tile([G // 2, BN], BF16, tag="vm")
                    tmpm = p4.tile([G // 2, 128], BF16, tag="tmpm")
                    for n in range(N):
                        js = [j for j in range(N) if A_np[n, j] != 0.0]
                        j0 = js[0]
                        nc.vector.tensor_scalar(
                            out=vm[:, n * 128:(n + 1) * 128],
                            in0=vbf[:, j0 * 128:(j0 + 1) * 128],
                            scalar1=float(A_np[n, j0]), scalar2=None,
                            op0=mybir.AluOpType.mult)
                        for j in js[1:]:
                            nc.vector.tensor_scalar(
                                out=tmpm[:],
                                in0=vbf[:, j * 128:(j + 1) * 128],
                                scalar1=float(A_np[n, j]), scalar2=None,
                                op0=mybir.AluOpType.mult)
                            nc.vector.tensor_add(
                                vm[:, n * 128:(n + 1) * 128],
                                vm[:, n * 128:(n + 1) * 128], tmpm[:])
                    hid = p4.tile([G // 2, BN], BF16, tag="hid")
                    nc.scalar.activation(hid[:], vm[:], AF.Silu,
                                         bias=mlpb1t[:, 0:1])
                    v1f = p4.tile([1, BN], F32, tag="v1f")
                    for c in range(4):
                        ohps = ps4.tile([1, CH], F32, tag="ohps")
                        nc.tensor.matmul(ohps[:], mlpw2t[:],
                                         hid[:, c * CH:c * CH + CH],
                                         start=True, stop=True)
                        nc.vector.tensor_scalar(
                            out=v1f[:, c * CH:c * CH + CH], in0=ohps[:],
                            scalar1=mlpb2t[0:1, 0:1], scalar2=None,
                            op0=mybir.AluOpType.add)
                    v15 = p4.tile([N + 1, BS], F32, tag="v15")
                    nc.vector.memset(v15[:], 1.0)
                    for n in range(N):
                        nc.sync.dma_start(v15[n:n + 1, :],
                                          v1f[0:1, n * BS:(n + 1) * BS])
                    fps = ps4.tile([NOUT, BS], F32, tag="fps")
                    nc.tensor.matmul(fps[:], poolwt[:], v15[:],
                                     start=True, stop=True)
                    outsb = p4.tile([NOUT, BS], F32, tag="outsb")
                    nc.vector.tensor_copy(outsb[:], fps[:])
                    for o in range(NOUT):
                        nc.sync.dma_start(out_e[:, o:o + 1],
                                          outsb[o:o + 1, :])
                    for dn, src in (("d_ubf", ubf), ("d_vm", vm),
                                    ("d_v1f", v1f)):
                        if dn in dump_es:
                            nc.sync.dma_start(dump_es[dn][:, :], src[:])

    nc.compile()
    return nc


def make_adjacency(edge_index):
    ei = np.asarray(edge_index)
    loops = np.arange(N, dtype=ei.dtype)
    row = np.concatenate([ei[0], loops])
    col = np.concatenate([ei[1], loops])
    deg = np.zeros(N, np.float32)
    np.add.at(deg, col, 1.0)
    dinv = np.where(deg > 0, deg ** -0.5, 0.0).astype(np.float32)
    norm = dinv[row] * dinv[col]
    A = np.zeros((N, N), np.float32)
    np.add.at(A, (col, row), norm)
    return A


def prep_inputs(inputs):
    """Host-side prep: per-core shards + weight layouts. Returns in_maps."""
    x = np.asarray(inputs["x"], np.float32)
    A = make_adjacency(inputs["edge_index"])
    c1 = 0.1 * np.asarray(inputs["sn_ws"], np.float32) + \
        np.asarray(inputs["sn_b1"], np.float32)
    W1p = np.asarray(inputs["sn_W1"], np.float32).reshape(N, T, F, SNH) \
        .transpose(1, 0, 2, 3).reshape(D, SNH)
    w1 = _bf16(np.vstack([W1p, c1[None, :]]))
    w2 = _bf16(np.vstack([np.asarray(inputs["sn_W2"], np.float32),
                          np.asarray(inputs["sn_b2"], np.float32)[None, :]]))
    wih = np.asarray(inputs["lstm_Wih"], np.float32).T      # [8, 512]
    whh = _bf16(np.asarray(inputs["lstm_Whh"], np.float32).T)  # [128, 512]
    wihx32 = np.zeros((32, 8, 512), np.float32)
    wihs32 = np.zeros((32, 8, 512), np.float32)
    for v in range(8):
        wihx32[v * 4:v * 4 + 4, v, :] = wih[0:4]
        wihs32[v * 4:v * 4 + 4, v, :] = wih[4:8]
    wihx = _bf16(np.tile(wihx32.reshape(32, 8 * 512), (4, 1)))
    wihs = _bf16(np.tile(wihs32.reshape(32, 8 * 512), (4, 1)))
    bg = _f32((np.asarray(inputs["lstm_bih"], np.float32) +
               np.asarray(inputs["lstm_bhh"], np.float32))
              .reshape(4, H).T)                              # [128, 4]
    gcnw = _bf16(inputs["gcn_W"])
    gcnb = _f32(np.asarray(inputs["gcn_b"]).reshape(G, 1))
    mlpw1 = _bf16(inputs["mlp_W1"])
    mlpb1 = _f32(np.asarray(inputs["mlp_b1"]).reshape(G // 2, 1))
    mlpw2 = _bf16(inputs["mlp_W2"])
    mlpb2 = _f32(np.asarray(inputs["mlp_b2"]).reshape(1, 1))
    poolw = _f32(np.vstack([np.asarray(inputs["pool_W"], np.float32),
                            np.asarray(inputs["pool_b"], np.float32)[None, :]]))
    ident = _bf16(np.eye(128, dtype=np.float32))

    shared = dict(w1=w1, w2=w2, whh=whh, wihx=wihx, wihs=wihs, bg=bg,
                  gcnw=gcnw, gcnb=gcnb, mlpw1=mlpw1, mlpb1=mlpb1,
                  mlpw2=mlpw2, mlpb2=mlpb2, poolw=poolw, ident=ident)
    in_maps = []
    for cidx in range(NCORES):
        xc = x[cidx * BS:(cidx + 1) * BS]            # [128, T, N, F]
        xflat = xc.reshape(BS, D)                    # (t,n,f) order
        xT = np.vstack([xflat.T, np.ones((1, BS), np.float32)])
        arr = xc.transpose(1, 3, 2, 0).reshape(T, F, BN)  # [t, f, (n,b)]
        arrp = np.zeros((NG2 * 32, F, BN), np.float32)
        arrp[:T] = arr
        xcx = arrp.reshape(NG2, 128, BN).reshape(NG2 * 128, BN)
        in_maps.append(dict(xt=_bf16(xT), xcx=_bf16(xcx), **shared))
    return in_maps, A


def kernel(**inputs):
    from concourse.bass_utils import run_bass_kernel_spmd
    in_maps, A = prep_inputs(inputs)
    nc = build_nc(A, reps=1)
    res = run_bass_kernel_spmd(nc, in_maps, core_ids=list(range(NCORES)))
    out = np.concatenate([res.results[c]["out"] for c in range(NCORES)], axis=0)
    return out.astype(np.float32)
